# revision 18
# baseline (speedup 1.0000x reference)
"""Trainium2 Bass kernel v3 for masked additive-attention pooling.

Reference math (per batch b):
    whhn = encoding @ W_h.T                            # [B, D]
    M    = tanh(X @ W_y.T + whhn[:, None, :])          # [B, T, D]
    a    = sigmoid(M @ w_a)                            # [B, T]
    e    = exp(a); den = sum(e * mask); w = e * mask / den
    out  = sum_t w[t] * X[t]                           # [B, D]

Sharding: data-parallel over batch B=32 across 8 cores (4 batches/core).
Weights replicated. Host does layout transforms only.

v3 changes vs v2 (183us measured):
  - apre (logits): 4-way col-tiled (tile_position) N=256 matmuls, one
    [128,32] wa-chunk stationary per strip, accumulating 8 e-chunks into
    4 PSUM row-strips -> ~120ns/group vs 2 DR N=512 MMs; kills the DR
    apre stream (13.8us) AND the t_cols K=1 transpose matmuls (~8us).
  - strip output goes through ONE DVE 32x32 block-transpose per half
    ([128,256] PSUM -> SBUF); a host-side token permutation within each
    512-token j-tile (c<->k swap) makes the transposed layout line up
    exactly with the pooling stationary columns. x-natural and mask are
    permuted identically on host; xt/z-stream order unchanged.
  - th stored as plain [128, eb, jl, 512] fp8 (no DR pair interleave).
  - apre groups emitted inline one eb behind the z stream (ACT latency
    hidden), so the epilogue tail after the last z group is short.
  - DMA issue spread across sync/scalar/vector/gpsimd queues with
    first-needed-first ordering: head shrinks and the early z stream no
    longer starves (HAM stays warm).
  - z DR stream unchanged: ~110us of fp8 DoubleRow matmuls = the PE
    hardware floor for this problem.
"""

import sys

if "/opt/trn_rl_repo" not in sys.path:
    sys.path.insert(0, "/opt/trn_rl_repo")

import numpy as np
import ml_dtypes

import concourse.bacc as bacc
import concourse.mybir as mybir
import concourse.tile as tile
from concourse.bass_utils import run_bass_kernel_spmd

F32 = mybir.dt.float32
BF16 = mybir.dt.bfloat16
FP8 = mybir.dt.float8e4
AF = mybir.ActivationFunctionType
DR = mybir.MatmulPerfMode.DoubleRow
MULT = mybir.AluOpType.mult
ADD = mybir.AluOpType.add

N_CORES = 8
B, T, D = 32, 2048, 1024
B_LOC = B // N_CORES          # 4 batches per core
NTOK = B_LOC * T              # 8192 tokens per core
TILE_T = 512                  # tokens per j-tile
NBT = NTOK // TILE_T          # 16 j-tiles
BT_PER_B = T // TILE_T        # 4 j-tiles per batch
CH = TILE_T // 128            # 4 128-token chunks per j-tile
KD = D // 128                 # 8 contraction chunks
EB = D // 128                 # 8 output-feature blocks
NH = NBT // 2                 # 8 halves (j-pairs)

_CACHE = {}


def build():
    nc = bacc.Bacc("TRN2", target_bir_lowering=False, debug=False,
                   num_devices=N_CORES)

    x = nc.dram_tensor("x", [NTOK, D], BF16, kind="ExternalInput").ap()
    xt = nc.dram_tensor("xt", [NBT, 128, KD * TILE_T], FP8,
                        kind="ExternalInput").ap()
    wyt = nc.dram_tensor("wyt", [EB, 128, KD * 128], FP8,
                         kind="ExternalInput").ap()
    wht = nc.dram_tensor("wht", [EB, 128, KD * 128], FP8,
                         kind="ExternalInput").ap()
    enc_cols = nc.dram_tensor("enc_cols", [128, KD * B_LOC], BF16,
                              kind="ExternalInput").ap()
    wa32 = nc.dram_tensor("wa32", [EB, 128, 32], BF16,
                          kind="ExternalInput").ap()
    mask_cols = nc.dram_tensor("mask_cols", [128, NTOK // 128], BF16,
                               kind="ExternalInput").ap()
    ones = nc.dram_tensor("ones", [128, 1], BF16, kind="ExternalInput").ap()
    hot4 = nc.dram_tensor("hot4", [128, 1], BF16, kind="ExternalInput").ap()
    out = nc.dram_tensor("out", [B_LOC, D], F32, kind="ExternalOutput").ap()

    x4 = x.rearrange("(j c p) d -> j p c d", p=128, c=CH)

    with tile.TileContext(nc) as tc:
        with tc.tile_pool(name="consts", bufs=1) as cp, \
             tc.tile_pool(name="wy", bufs=1) as wyp, \
             tc.tile_pool(name="xnat", bufs=8) as xp, \
             tc.tile_pool(name="xt", bufs=4) as xtp, \
             tc.tile_pool(name="th", bufs=2) as thp, \
             tc.tile_pool(name="small", bufs=2) as smp, \
             tc.tile_pool(name="mps", bufs=1, space="PSUM") as psum:

            state = {}
            pending = []

            def pop1():
                if pending:
                    pending.pop(0)()

            def load_xt(j, split=1, eng=None):
                eng = eng or nc.gpsimd
                t = xtp.tile([128, KD * TILE_T], FP8, tag="xt",
                             name=f"xt_{j}")
                w = KD * TILE_T // split
                for s in range(split):
                    eng.dma_start(
                        t[:, s * w:(s + 1) * w],
                        xt[j][:, s * w:(s + 1) * w])
                state[("xt", j)] = t

            def load_xnat(j, eng=None):
                eng = eng or nc.sync
                t = xp.tile([128, CH * D], BF16, tag="xn", name=f"x_{j}")
                eng.dma_start(
                    t[:].rearrange("p (c d) -> p c d", c=CH), x4[j])
                state[("xn", j)] = t

            # ---- phase 0: DMAs spread across queues, first-needed-first.
            whp_cm = tc.tile_pool(name="wh", bufs=1)
            whp = whp_cm.__enter__()
            xt_t0 = xtp.tile([128, KD * TILE_T], FP8, tag="xt", name="xt_0")
            xt_t1 = xtp.tile([128, KD * TILE_T], FP8, tag="xt", name="xt_1")
            state[("xt", 0)] = xt_t0
            state[("xt", 1)] = xt_t1
            wy_sb = [wyp.tile([128, KD * 128], FP8, tag=f"wy{eb}",
                              name=f"wy_{eb}") for eb in range(EB)]
            wh_sb = [whp.tile([128, KD * 128], FP8, tag=f"wh{eb}",
                              name=f"wh_{eb}") for eb in range(EB)]
            enc_sb = cp.tile([128, KD * B_LOC], BF16)
            ones_sb = cp.tile([128, 1], BF16)
            half_sb = cp.tile([128, 1], F32)
            nc.vector.memset(half_sb[:], 0.5)
            wa_sb = [cp.tile([128, 32], BF16, name=f"wa_{eb}")
                     for eb in range(EB)]
            mask_sb = cp.tile([128, NTOK // 128], BF16)
            whhn_sb = cp.tile([128, EB * B_LOC], F32)
            hot_sb = cp.tile([128, 1], BF16)

            # sync queue: xt0/xt1 quarters (the z-stream critical path)
            QW = KD * TILE_T // 4
            for s in range(4):
                nc.sync.dma_start(xt_t0[:, s * QW:(s + 1) * QW],
                                  xt[0][:, s * QW:(s + 1) * QW])
                nc.sync.dma_start(xt_t1[:, s * QW:(s + 1) * QW],
                                  xt[1][:, s * QW:(s + 1) * QW])
            # scalar queue: wy weights (one needed every ~2.1us), then the
            # first x-natural tiles (needed at h0's pooling, ~+18us)
            for s in range(EB):
                nc.scalar.dma_start(wy_sb[s][:], wyt[s])
            load_xnat(0, eng=nc.scalar)
            load_xnat(1, eng=nc.scalar)
            # gpsimd queue: tiny apre/whhn consts first, then wh trickle
            # interleaved with the h1 xt tiles so neither starves
            nc.gpsimd.dma_start(enc_sb[:], enc_cols[:])
            for s in range(EB):
                nc.gpsimd.dma_start(wa_sb[s][:], wa32[s])
            nc.gpsimd.dma_start(ones_sb[:], ones[:])
            nc.gpsimd.dma_start(hot_sb[:], hot4[:])
            for s in range(3):
                nc.gpsimd.dma_start(wh_sb[s][:], wht[s])
            load_xt(2, eng=nc.gpsimd)
            nc.gpsimd.dma_start(wh_sb[3][:], wht[3])
            load_xt(3, eng=nc.gpsimd)
            for s in range(4, EB):
                nc.gpsimd.dma_start(wh_sb[s][:], wht[s])
            nc.gpsimd.dma_start(mask_sb[:], mask_cols[:])
            load_xnat(2, eng=nc.sync)
            load_xnat(3, eng=nc.sync)

            # ---- whhn: one eb at a time (interleaved into early z) ----
            def emit_whhn(eb):
                def fn():
                    php = psum.tile([128, B_LOC], F32, tag="small", bufs=1,
                                    name=f"php_{eb}")
                    for k in range(KD):
                        nc.tensor.matmul(
                            php[:], wh_sb[eb][:, k * 128:(k + 1) * 128],
                            enc_sb[:, k * B_LOC:(k + 1) * B_LOC],
                            start=(k == 0), stop=(k == KD - 1))
                    nc.vector.tensor_copy(
                        whhn_sb[:, eb * B_LOC:(eb + 1) * B_LOC], php[:])
                return fn

            # ---- z + tanh for one (half, eb): both j-tiles share LDW ----
            def get_th(h):
                key = ("th", h)
                if key not in state:
                    state[key] = thp.tile([128, EB * 2 * 512], FP8,
                                          tag="th", name=f"th_{h}")
                return state[key]

            def emit_z(h, eb, mid=None):
                j0 = 2 * h
                b = j0 // BT_PER_B
                zp = psum.tile([128, 2 * 512], F32, tag="z", bufs=2,
                               name=f"z_{h}_{eb}")
                xt0 = state[("xt", j0)]
                xt1 = state[("xt", j0 + 1)]
                for q in range(KD // 2):
                    w_ap = wy_sb[eb][:, q * 256:(q + 1) * 256].rearrange(
                        "p (i m) -> p i m", i=2)
                    for jl, xt_t in ((0, xt0), (1, xt1)):
                        mm = nc.tensor.matmul(
                            zp[:, jl * 512:(jl + 1) * 512], w_ap,
                            xt_t[:, 2 * q * TILE_T:(2 * q + 2) * TILE_T]
                            .rearrange("p (i n) -> p i n", i=2),
                            start=(q == 0), stop=(q == KD // 2 - 1),
                            perf_mode=DR)
                        if jl == 1:
                            mm.ldweights = False
                if mid is not None:
                    mid()
                # tanh over both j-tiles at once; bias whhn[:, eb, b]
                th_t = get_th(h)
                thv = th_t[:].rearrange("p (e jl n) -> p e jl n",
                                        e=EB, jl=2)
                nc.scalar.activation(
                    thv[:, eb], zp[:].rearrange("p (jl n) -> p jl n", jl=2),
                    AF.Tanh, scale=1.0 / 32.0,
                    bias=whhn_sb[:, eb * B_LOC + b:eb * B_LOC + b + 1])

            # ---- apre: 4-way col-tiled strips, one group per (h, eb).
            # Full-bank tile (exclusive has_written domain): the single
            # start=True on the first MM clears the bank; the other strips
            # first-touch-overwrite, then everything accumulates.
            def get_aps(h):
                key = ("aps", h)
                if key not in state:
                    t = psum.tile([128, 512], F32, tag="aps",
                                  bufs=1, name=f"aps_{h}")
                    nc.vector.memset(t[:, 0:256], 0.0)
                    state[key] = t
                return state[key]

            def emit_apre(h, eb):
                th_t = state[("th", h)]
                thv = th_t[:].rearrange("p (e jl n) -> p e jl n",
                                        e=EB, jl=2)
                aps = get_aps(h)
                apsv = aps[:, 0:256].rearrange("p (jl n) -> p jl n", jl=2)
                for c in range(4):
                    nc.tensor.matmul(
                        apsv[32 * c:32 * (c + 1), :, :],
                        wa_sb[eb][:],
                        thv[:, eb, :, 128 * c:128 * (c + 1)],
                        start=False, stop=(eb == EB - 1),
                        tile_position=(0, 32 * c),
                        skip_group_check=True)
                if eb == EB - 1:
                    state.pop(("th", h))

            # ---- per-half epilogue, interleaved between z groups ----
            def get_ew(b):
                key = ("ew", b)
                if key not in state:
                    state[key] = smp.tile([128, 4 * CH], BF16, tag="ew",
                                          name=f"ew_{b}")
                return state[key]

            def queue_h_epilogue(h):
                b = h // 2
                hh = h % 2       # which j-pair of the batch

                def t_trans():
                    aps = state.pop(("aps", h))
                    apct = smp.tile([128, 256], F32, tag="apct",
                                    name=f"apct_{h}")
                    nc.vector.transpose(apct[:], aps[:, 0:256])
                    state[("apct", h)] = apct

                def t_act():
                    apct = state.pop(("apct", h))
                    apcv = apct[:].rearrange("p (m e) -> p m e", m=8)
                    tj = smp.tile([128, 8], F32, tag="tj", name=f"tj_{h}")
                    nc.scalar.activation(tj[:], apcv[:, :, 0:1], AF.Tanh,
                                         scale=0.5)
                    ej = smp.tile([128, 8], F32, tag="ej", name=f"ej_{h}")
                    nc.scalar.activation(ej[:], tj[:], AF.Exp,
                                         bias=half_sb[:], scale=0.5)
                    ew = get_ew(b)
                    nc.vector.tensor_mul(
                        ew[:, hh * 8:(hh + 1) * 8], ej[:],
                        mask_sb[:, b * 16 + hh * 8:b * 16 + (hh + 1) * 8])

                def t_den():
                    ew = state[("ew", b)]
                    acc = smp.tile([128, 1], F32, tag="acc",
                                   name=f"acc_{b}")
                    nc.vector.tensor_reduce(
                        acc[:], ew[:], mybir.AxisListType.XYZW, ADD)
                    accb = smp.tile([128, 1], BF16, tag="accb",
                                    name=f"accb_{b}")
                    nc.vector.tensor_copy(accb[:], acc[:])
                    den = psum.tile([1, 1], F32, tag="small", bufs=1,
                                    name=f"den_{b}")
                    nc.tensor.matmul(den[0:1, :], ones_sb[:, 0:1],
                                     accb[:], start=True, stop=True)
                    rec = smp.tile([1, 1], F32, tag="rec", name=f"rec_{b}")
                    nc.vector.reciprocal(rec[:], den[0:1, :])
                    state[("rec", b)] = rec

                def t_pool(dn):
                    def fn():
                        ew = state[("ew", b)]
                        if hh == 0:
                            state[("num", b, dn)] = psum.tile(
                                [128, 512], F32, tag="num", bufs=2,
                                name=f"num_{b}_{dn}")
                            nc.vector.memset(state[("num", b, dn)][:], 0.0)
                        num = state[("num", b, dn)]
                        for jl in range(2):
                            j = 2 * h + jl
                            xn = state[("xn", j)]
                            for k in range(CH):
                                col = (hh * 2 + jl) * CH + k
                                pos = k * 32
                                nc.tensor.matmul(
                                    num[pos:pos + 1, :],
                                    ew[:, col:col + 1],
                                    xn[:, k * D + dn * 512:
                                       k * D + (dn + 1) * 512],
                                    start=(col == 0),
                                    stop=(hh == 1 and jl == 1),
                                    tile_position=(0, pos),
                                    skip_group_check=True)
                            if dn == 1:
                                state.pop(("xn", j))
                    return fn

                def t_scale(dn):
                    def fn():
                        num = state.pop(("num", b, dn))
                        nsb = smp.tile([128, 512], BF16, tag="nsb",
                                       name=f"nsb_{b}_{dn}")
                        nc.vector.tensor_copy(nsb[:], num[:])
                        ns = psum.tile([1, 512], F32, tag="small", bufs=1,
                                       name=f"ns_{b}_{dn}")
                        nc.tensor.matmul(ns[0:1, :], hot_sb[:], nsb[:],
                                         start=True, stop=True)
                        rec = state[("rec", b)]
                        if ("ob", b) not in state:
                            state[("ob", b)] = smp.tile(
                                [1, D], F32, tag="ob", name=f"ob_{b}")
                        ob = state[("ob", b)]
                        nc.vector.tensor_scalar_mul(
                            ob[:, dn * 512:(dn + 1) * 512], ns[0:1, :],
                            rec[:])
                        if dn == 1:
                            state.pop(("rec", b))
                            state.pop(("ew", b))
                            ob = state.pop(("ob", b))
                            nc.sync.dma_start(out[b:b + 1, :], ob[:])
                    return fn

                pending.append(t_trans)
                pending.append(t_act)
                if hh == 1:
                    pending.append(t_den)
                pending.append(t_pool(0))
                pending.append(t_pool(1))
                if hh == 1:
                    pending.append(t_scale(0))
                    pending.append(t_scale(1))

            # ---- main loop over halves ----
            for h in range(NH):
                j0 = 2 * h
                if j0 + 4 < NBT:
                    load_xt(j0 + 4)
                if j0 + 5 < NBT:
                    load_xt(j0 + 5)
                if j0 + 4 < NBT:
                    load_xnat(j0 + 4)
                if j0 + 5 < NBT:
                    load_xnat(j0 + 5)
                for eb in range(EB):
                    if h == 0:
                        emit_z(h, eb, mid=emit_whhn(eb))
                    else:
                        emit_z(h, eb)
                    # apre one group behind the z stream (ACT latency)
                    if eb > 0:
                        emit_apre(h, eb - 1)
                    elif h > 0:
                        emit_apre(h - 1, EB - 1)
                    if h > 0:
                        pop1()
                        if len(pending) > 5:
                            pop1()
                queue_h_epilogue(h)
                if h == 0:
                    whp_cm.__exit__(None, None, None)
            emit_apre(NH - 1, EB - 1)
            while pending:
                pop1()

    nc.compile()
    return nc


def _host_pack(full_input, encoding, mask, W_h, W_y, w_a):
    """Per-core input maps (layout transforms / casts only)."""
    wyT = np.ascontiguousarray(W_y.T)  # [d, e]
    whT = np.ascontiguousarray(W_h.T)
    wyt_rows = np.empty((EB, 128, KD * 128), ml_dtypes.float8_e4m3)
    wht_rows = np.empty((EB, 128, KD * 128), ml_dtypes.float8_e4m3)
    for eb in range(EB):
        for k in range(KD):
            wyt_rows[eb, :, k * 128:(k + 1) * 128] = (
                32.0 * wyT[k * 128:(k + 1) * 128, eb * 128:(eb + 1) * 128])
            wht_rows[eb, :, k * 128:(k + 1) * 128] = (
                32.0 * whT[k * 128:(k + 1) * 128, eb * 128:(eb + 1) * 128])
    # per-strip stationary wa chunks: col 0 = w_a chunk, cols 1-31 zero
    wa_pack = np.zeros((EB, 128, 32), np.float32)
    wa_pack[:, :, 0] = w_a.reshape(EB, 128)
    wa_pack = wa_pack.astype(ml_dtypes.bfloat16)
    ones = np.ones((128, 1), ml_dtypes.bfloat16)
    hot4 = np.zeros((128, 1), np.float32)
    hot4[[0, 32, 64, 96]] = 1.0
    hot4 = hot4.astype(ml_dtypes.bfloat16)

    in_maps = []
    for ci in range(N_CORES):
        sl = slice(ci * B_LOC, (ci + 1) * B_LOC)
        xf = np.ascontiguousarray(
            full_input[sl].reshape(NTOK, D).astype(np.float32))
        # token permutation within each j-tile: stored (k, cc, i) holds
        # logical (cc, k, i) so the DVE block transpose of the apre strips
        # lands exactly on the pooling stationary layout.
        xperm = np.ascontiguousarray(
            xf.reshape(NBT, 4, 4, 32, D).transpose(0, 2, 1, 3, 4)
            .reshape(NTOK, D))
        x_i = xperm.astype(ml_dtypes.bfloat16)
        xt_i = np.ascontiguousarray(
            xf.T.astype(ml_dtypes.float8_e4m3)      # [D, NTOK], logical
            .reshape(KD, 128, NBT, TILE_T)
            .transpose(2, 1, 0, 3)                  # [j, p, k, t]
            .reshape(NBT, 128, KD * TILE_T))
        enc_i = np.ascontiguousarray(
            (1.0 / 32.0) * encoding[sl].T.reshape(KD, 128, B_LOC)
            .transpose(1, 0, 2)
            .reshape(128, KD * B_LOC)).astype(ml_dtypes.bfloat16)
        mperm = (np.ascontiguousarray(mask[sl]).reshape(NTOK)
                 .reshape(NBT, 4, 4, 32).transpose(0, 2, 1, 3)
                 .reshape(NTOK))
        mask_i = np.ascontiguousarray(
            mperm.reshape(NTOK // 128, 128).T).astype(ml_dtypes.bfloat16)
        in_maps.append({
            "x": x_i, "xt": xt_i, "wyt": wyt_rows, "wht": wht_rows,
            "enc_cols": enc_i, "wa32": wa_pack, "mask_cols": mask_i,
            "ones": ones, "hot4": hot4,
        })
    return in_maps


def run(inputs, trace=False):
    if "nc" not in _CACHE:
        _CACHE["nc"] = build()
    nc = _CACHE["nc"]
    in_maps = _host_pack(**inputs)
    res = run_bass_kernel_spmd(nc, in_maps, core_ids=list(range(N_CORES)),
                               trace=trace)
    out = np.concatenate([res.results[i]["out"] for i in range(N_CORES)],
                         axis=0)
    return out, res


def kernel(**inputs):
    inputs = {k: np.asarray(v) for k, v in inputs.items()}
    out, _ = run(inputs, trace=False)
    return out


# revision 19
# speedup vs baseline: 1.2049x; 1.2049x over previous
"""Trainium2 Bass kernel v3 for masked additive-attention pooling.

Reference math (per batch b):
    whhn = encoding @ W_h.T                            # [B, D]
    M    = tanh(X @ W_y.T + whhn[:, None, :])          # [B, T, D]
    a    = sigmoid(M @ w_a)                            # [B, T]
    e    = exp(a); den = sum(e * mask); w = e * mask / den
    out  = sum_t w[t] * X[t]                           # [B, D]

Sharding: data-parallel over batch B=32 across 8 cores (4 batches/core).
Weights replicated. Host does layout transforms only.

v3 changes vs v2 (183us measured):
  - apre (logits): 4-way col-tiled (tile_position) N=256 matmuls, one
    [128,32] wa-chunk stationary per strip, accumulating 8 e-chunks into
    4 PSUM row-strips -> ~120ns/group vs 2 DR N=512 MMs; kills the DR
    apre stream (13.8us) AND the t_cols K=1 transpose matmuls (~8us).
  - strip output goes through ONE DVE 32x32 block-transpose per half
    ([128,256] PSUM -> SBUF); a host-side token permutation within each
    512-token j-tile (c<->k swap) makes the transposed layout line up
    exactly with the pooling stationary columns. x-natural and mask are
    permuted identically on host; xt/z-stream order unchanged.
  - th stored as plain [128, eb, jl, 512] fp8 (no DR pair interleave).
  - apre groups emitted inline one eb behind the z stream (ACT latency
    hidden), so the epilogue tail after the last z group is short.
  - DMA issue spread across sync/scalar/vector/gpsimd queues with
    first-needed-first ordering: head shrinks and the early z stream no
    longer starves (HAM stays warm).
  - z DR stream unchanged: ~110us of fp8 DoubleRow matmuls = the PE
    hardware floor for this problem.
"""

import sys

if "/opt/trn_rl_repo" not in sys.path:
    sys.path.insert(0, "/opt/trn_rl_repo")

import numpy as np
import ml_dtypes

import concourse.bacc as bacc
import concourse.mybir as mybir
import concourse.tile as tile
from concourse.bass_utils import run_bass_kernel_spmd

F32 = mybir.dt.float32
BF16 = mybir.dt.bfloat16
FP8 = mybir.dt.float8e4
AF = mybir.ActivationFunctionType
DR = mybir.MatmulPerfMode.DoubleRow
MULT = mybir.AluOpType.mult
ADD = mybir.AluOpType.add

N_CORES = 8
B, T, D = 32, 2048, 1024
B_LOC = B // N_CORES          # 4 batches per core
NTOK = B_LOC * T              # 8192 tokens per core
TILE_T = 512                  # tokens per j-tile
NBT = NTOK // TILE_T          # 16 j-tiles
BT_PER_B = T // TILE_T        # 4 j-tiles per batch
CH = TILE_T // 128            # 4 128-token chunks per j-tile
KD = D // 128                 # 8 contraction chunks
EB = D // 128                 # 8 output-feature blocks
NH = NBT // 2                 # 8 halves (j-pairs)

_CACHE = {}


def build():
    nc = bacc.Bacc("TRN2", target_bir_lowering=False, debug=False,
                   num_devices=N_CORES)

    x = nc.dram_tensor("x", [NTOK, D], BF16, kind="ExternalInput").ap()
    xt = nc.dram_tensor("xt", [NBT, 128, KD * TILE_T], FP8,
                        kind="ExternalInput").ap()
    wyt = nc.dram_tensor("wyt", [EB, 128, KD * 128], FP8,
                         kind="ExternalInput").ap()
    wht = nc.dram_tensor("wht", [EB, 128, KD * 128], FP8,
                         kind="ExternalInput").ap()
    CW = KD * B_LOC + EB * 32 + 1 + 1 + NTOK // 128
    consts = nc.dram_tensor("consts", [128, CW], BF16,
                            kind="ExternalInput").ap()
    out = nc.dram_tensor("out", [B_LOC, D], F32, kind="ExternalOutput").ap()

    x4 = x.rearrange("(j c p) d -> j p c d", p=128, c=CH)

    with tile.TileContext(nc) as tc:
        with tc.tile_pool(name="consts", bufs=1) as cp, \
             tc.tile_pool(name="wy", bufs=1) as wyp, \
             tc.tile_pool(name="xnat", bufs=8) as xp, \
             tc.tile_pool(name="xt", bufs=4) as xtp, \
             tc.tile_pool(name="th", bufs=2) as thp, \
             tc.tile_pool(name="small", bufs=2) as smp, \
             tc.tile_pool(name="mps", bufs=1, space="PSUM") as psum:

            state = {}
            pending = []

            def pop1():
                if pending:
                    pending.pop(0)()

            def load_xt(j, split=1, eng=None):
                eng = eng or nc.gpsimd
                t = xtp.tile([128, KD * TILE_T], FP8, tag="xt",
                             name=f"xt_{j}")
                w = KD * TILE_T // split
                for s in range(split):
                    eng.dma_start(
                        t[:, s * w:(s + 1) * w],
                        xt[j][:, s * w:(s + 1) * w])
                state[("xt", j)] = t

            def load_xnat(j, eng=None):
                eng = eng or nc.sync
                t = xp.tile([128, CH * D], BF16, tag="xn", name=f"x_{j}")
                eng.dma_start(
                    t[:].rearrange("p (c d) -> p c d", c=CH), x4[j])
                state[("xn", j)] = t

            # ---- phase 0: DMAs spread across queues, first-needed-first.
            whp_cm = tc.tile_pool(name="wh", bufs=1)
            whp = whp_cm.__enter__()
            xt_t0 = xtp.tile([128, KD * TILE_T], FP8, tag="xt", name="xt_0")
            xt_t1 = xtp.tile([128, KD * TILE_T], FP8, tag="xt", name="xt_1")
            state[("xt", 0)] = xt_t0
            state[("xt", 1)] = xt_t1
            wy_sb = [wyp.tile([128, KD * 128], FP8, tag=f"wy{eb}",
                              name=f"wy_{eb}") for eb in range(EB)]
            wh_sb = [whp.tile([128, KD * 128], FP8, tag=f"wh{eb}",
                              name=f"wh_{eb}") for eb in range(EB)]
            consts_sb = cp.tile([128, CW], BF16)
            enc_sb = consts_sb[:, 0:KD * B_LOC]
            wa_sb = [consts_sb[:, KD * B_LOC + 32 * eb:
                               KD * B_LOC + 32 * (eb + 1)]
                     for eb in range(EB)]
            _o0 = KD * B_LOC + EB * 32
            ones_sb = consts_sb[:, _o0:_o0 + 1]
            hot_sb = consts_sb[:, _o0 + 1:_o0 + 2]
            mask_sb = consts_sb[:, _o0 + 2:_o0 + 2 + NTOK // 128]
            half_sb = cp.tile([128, 1], F32)
            nc.vector.memset(half_sb[:], 0.5)
            whhn_sb = cp.tile([128, EB * B_LOC], F32)

            # sync queue: xt0/xt1 quarters (the z-stream critical path)
            QW = KD * TILE_T // 4
            for s in range(4):
                nc.sync.dma_start(xt_t0[:, s * QW:(s + 1) * QW],
                                  xt[0][:, s * QW:(s + 1) * QW])
                nc.sync.dma_start(xt_t1[:, s * QW:(s + 1) * QW],
                                  xt[1][:, s * QW:(s + 1) * QW])
            # scalar queue: wy weights (one needed every ~2.1us), then the
            # first x-natural tiles (needed at h0's pooling, ~+18us)
            for s in range(EB):
                nc.scalar.dma_start(wy_sb[s][:], wyt[s])
            load_xnat(0, eng=nc.scalar)
            load_xnat(1, eng=nc.scalar)
            # gpsimd queue: ONE packed consts DMA, then wh trickle
            # interleaved with the h1 xt tiles so neither starves
            nc.gpsimd.dma_start(consts_sb[:], consts[:])
            for s in range(3):
                nc.gpsimd.dma_start(wh_sb[s][:], wht[s])
            load_xt(2, eng=nc.gpsimd)
            nc.gpsimd.dma_start(wh_sb[3][:], wht[3])
            load_xt(3, eng=nc.gpsimd)
            for s in range(4, EB):
                nc.gpsimd.dma_start(wh_sb[s][:], wht[s])
            load_xnat(2, eng=nc.sync)
            load_xnat(3, eng=nc.sync)

            # ---- whhn: one eb at a time (interleaved into early z) ----
            def emit_whhn(eb):
                def fn():
                    php = psum.tile([128, B_LOC], F32, tag="small", bufs=1,
                                    name=f"php_{eb}")
                    for k in range(KD):
                        nc.tensor.matmul(
                            php[:], wh_sb[eb][:, k * 128:(k + 1) * 128],
                            enc_sb[:, k * B_LOC:(k + 1) * B_LOC],
                            start=(k == 0), stop=(k == KD - 1))
                    nc.vector.tensor_copy(
                        whhn_sb[:, eb * B_LOC:(eb + 1) * B_LOC], php[:])
                return fn

            # ---- z + tanh for one (half, eb): both j-tiles share LDW ----
            def get_th(h):
                key = ("th", h)
                if key not in state:
                    state[key] = thp.tile([128, EB * 2 * 512], FP8,
                                          tag="th", name=f"th_{h}")
                return state[key]

            def emit_z(h, eb, mid=None):
                j0 = 2 * h
                b = j0 // BT_PER_B
                zp = psum.tile([128, 2 * 512], F32, tag="z", bufs=2,
                               name=f"z_{h}_{eb}")
                xt0 = state[("xt", j0)]
                xt1 = state[("xt", j0 + 1)]
                for q in range(KD // 2):
                    w_ap = wy_sb[eb][:, q * 256:(q + 1) * 256].rearrange(
                        "p (i m) -> p i m", i=2)
                    for jl, xt_t in ((0, xt0), (1, xt1)):
                        mm = nc.tensor.matmul(
                            zp[:, jl * 512:(jl + 1) * 512], w_ap,
                            xt_t[:, 2 * q * TILE_T:(2 * q + 2) * TILE_T]
                            .rearrange("p (i n) -> p i n", i=2),
                            start=(q == 0), stop=(q == KD // 2 - 1),
                            perf_mode=DR)
                        if jl == 1:
                            mm.ldweights = False
                if mid is not None:
                    mid()
                # tanh over both j-tiles at once; bias whhn[:, eb, b]
                th_t = get_th(h)
                thv = th_t[:].rearrange("p (e jl n) -> p e jl n",
                                        e=EB, jl=2)
                nc.scalar.activation(
                    thv[:, eb], zp[:].rearrange("p (jl n) -> p jl n", jl=2),
                    AF.Tanh, scale=1.0 / 32.0,
                    bias=whhn_sb[:, eb * B_LOC + b:eb * B_LOC + b + 1])

            # ---- apre: 4-way col-tiled strips, one group per (h, eb).
            # Full-bank tile (exclusive has_written domain): the single
            # start=True on the first MM clears the bank; the other strips
            # first-touch-overwrite, then everything accumulates.
            def get_aps(h):
                key = ("aps", h)
                if key not in state:
                    t = psum.tile([128, 512], F32, tag="aps",
                                  bufs=1, name=f"aps_{h}")
                    nc.vector.memset(t[:, 0:256], 0.0)
                    state[key] = t
                return state[key]

            def emit_apre(h, eb):
                th_t = state[("th", h)]
                thv = th_t[:].rearrange("p (e jl n) -> p e jl n",
                                        e=EB, jl=2)
                aps = get_aps(h)
                apsv = aps[:, 0:256].rearrange("p (jl n) -> p jl n", jl=2)
                for c in range(4):
                    nc.tensor.matmul(
                        apsv[32 * c:32 * (c + 1), :, :],
                        wa_sb[eb],
                        thv[:, eb, :, 128 * c:128 * (c + 1)],
                        start=False, stop=(eb == EB - 1),
                        tile_position=(0, 32 * c),
                        skip_group_check=True)
                if eb == EB - 1:
                    state.pop(("th", h))

            # ---- per-half epilogue, interleaved between z groups ----
            def get_ew(b):
                key = ("ew", b)
                if key not in state:
                    state[key] = smp.tile([128, 4 * CH], BF16, tag="ew",
                                          name=f"ew_{b}")
                return state[key]

            def queue_h_epilogue(h):
                b = h // 2
                hh = h % 2       # which j-pair of the batch

                def t_trans():
                    aps = state.pop(("aps", h))
                    apct = smp.tile([128, 256], F32, tag="apct",
                                    name=f"apct_{h}")
                    nc.vector.transpose(apct[:], aps[:, 0:256])
                    state[("apct", h)] = apct

                def t_act():
                    apct = state.pop(("apct", h))
                    apcv = apct[:].rearrange("p (m e) -> p m e", m=8)
                    tj = smp.tile([128, 8], F32, tag="tj", name=f"tj_{h}")
                    nc.scalar.activation(tj[:], apcv[:, :, 0:1], AF.Tanh,
                                         scale=0.5)
                    ej = smp.tile([128, 8], F32, tag="ej", name=f"ej_{h}")
                    nc.scalar.activation(ej[:], tj[:], AF.Exp,
                                         bias=half_sb[:], scale=0.5)
                    ew = get_ew(b)
                    nc.vector.tensor_mul(
                        ew[:, hh * 8:(hh + 1) * 8], ej[:],
                        mask_sb[:, b * 16 + hh * 8:b * 16 + (hh + 1) * 8])

                def t_den():
                    ew = state[("ew", b)]
                    acc = smp.tile([128, 1], F32, tag="acc",
                                   name=f"acc_{b}")
                    nc.vector.tensor_reduce(
                        acc[:], ew[:], mybir.AxisListType.XYZW, ADD)
                    accb = smp.tile([128, 1], BF16, tag="accb",
                                    name=f"accb_{b}")
                    nc.vector.tensor_copy(accb[:], acc[:])
                    den = psum.tile([1, 1], F32, tag="small", bufs=1,
                                    name=f"den_{b}")
                    nc.tensor.matmul(den[0:1, :], ones_sb,
                                     accb[:], start=True, stop=True)
                    rec = smp.tile([1, 1], F32, tag="rec", name=f"rec_{b}")
                    nc.vector.reciprocal(rec[:], den[0:1, :])
                    state[("rec", b)] = rec

                def t_pool(dn):
                    def fn():
                        ew = state[("ew", b)]
                        if hh == 0:
                            state[("num", b, dn)] = psum.tile(
                                [128, 512], F32, tag="num", bufs=2,
                                name=f"num_{b}_{dn}")
                            nc.vector.memset(state[("num", b, dn)][:], 0.0)
                        num = state[("num", b, dn)]
                        for jl in range(2):
                            j = 2 * h + jl
                            xn = state[("xn", j)]
                            for k in range(CH):
                                col = (hh * 2 + jl) * CH + k
                                pos = k * 32
                                nc.tensor.matmul(
                                    num[pos:pos + 1, :],
                                    ew[:, col:col + 1],
                                    xn[:, k * D + dn * 512:
                                       k * D + (dn + 1) * 512],
                                    start=(col == 0),
                                    stop=(hh == 1 and jl == 1),
                                    tile_position=(0, pos),
                                    skip_group_check=True)
                            if dn == 1:
                                state.pop(("xn", j))
                    return fn

                def t_scale(dn):
                    def fn():
                        num = state.pop(("num", b, dn))
                        nsb = smp.tile([128, 512], BF16, tag="nsb",
                                       name=f"nsb_{b}_{dn}")
                        nc.vector.tensor_copy(nsb[:], num[:])
                        ns = psum.tile([1, 512], F32, tag="small", bufs=1,
                                       name=f"ns_{b}_{dn}")
                        nc.tensor.matmul(ns[0:1, :], hot_sb, nsb[:],
                                         start=True, stop=True)
                        rec = state[("rec", b)]
                        if ("ob", b) not in state:
                            state[("ob", b)] = smp.tile(
                                [1, D], F32, tag="ob", name=f"ob_{b}")
                        ob = state[("ob", b)]
                        nc.vector.tensor_scalar_mul(
                            ob[:, dn * 512:(dn + 1) * 512], ns[0:1, :],
                            rec[:])
                        if dn == 1:
                            state.pop(("rec", b))
                            state.pop(("ew", b))
                            ob = state.pop(("ob", b))
                            nc.sync.dma_start(out[b:b + 1, :], ob[:])
                    return fn

                pending.append(t_trans)
                pending.append(t_act)
                if hh == 1:
                    pending.append(t_den)
                pending.append(t_pool(0))
                pending.append(t_pool(1))
                if hh == 1:
                    pending.append(t_scale(0))
                    pending.append(t_scale(1))

            # ---- main loop over halves ----
            for h in range(NH):
                j0 = 2 * h
                if j0 + 4 < NBT:
                    load_xt(j0 + 4)
                if j0 + 5 < NBT:
                    load_xt(j0 + 5)
                if j0 + 4 < NBT:
                    load_xnat(j0 + 4)
                if j0 + 5 < NBT:
                    load_xnat(j0 + 5)
                for eb in range(EB):
                    if h == 0:
                        emit_z(h, eb, mid=emit_whhn(eb))
                    else:
                        emit_z(h, eb)
                    # apre one group behind the z stream (ACT latency)
                    if eb > 0:
                        emit_apre(h, eb - 1)
                    elif h > 0:
                        emit_apre(h - 1, EB - 1)
                    if h > 0:
                        pop1()
                        if len(pending) > 5:
                            pop1()
                queue_h_epilogue(h)
                if h == 0:
                    whp_cm.__exit__(None, None, None)
            emit_apre(NH - 1, EB - 1)
            while pending:
                pop1()

    nc.compile()
    return nc


def _host_pack(full_input, encoding, mask, W_h, W_y, w_a):
    """Per-core input maps (layout transforms / casts only)."""
    wyT = np.ascontiguousarray(W_y.T)  # [d, e]
    whT = np.ascontiguousarray(W_h.T)
    wyt_rows = np.empty((EB, 128, KD * 128), ml_dtypes.float8_e4m3)
    wht_rows = np.empty((EB, 128, KD * 128), ml_dtypes.float8_e4m3)
    for eb in range(EB):
        for k in range(KD):
            wyt_rows[eb, :, k * 128:(k + 1) * 128] = (
                32.0 * wyT[k * 128:(k + 1) * 128, eb * 128:(eb + 1) * 128])
            wht_rows[eb, :, k * 128:(k + 1) * 128] = (
                32.0 * whT[k * 128:(k + 1) * 128, eb * 128:(eb + 1) * 128])
    CW = KD * B_LOC + EB * 32 + 1 + 1 + NTOK // 128

    in_maps = []
    for ci in range(N_CORES):
        sl = slice(ci * B_LOC, (ci + 1) * B_LOC)
        xf = np.ascontiguousarray(
            full_input[sl].reshape(NTOK, D).astype(np.float32))
        # token permutation within each j-tile: stored (k, cc, i) holds
        # logical (cc, k, i) so the DVE block transpose of the apre strips
        # lands exactly on the pooling stationary layout.
        xperm = np.ascontiguousarray(
            xf.reshape(NBT, 4, 4, 32, D).transpose(0, 2, 1, 3, 4)
            .reshape(NTOK, D))
        x_i = xperm.astype(ml_dtypes.bfloat16)
        xt_i = np.ascontiguousarray(
            xf.T.astype(ml_dtypes.float8_e4m3)      # [D, NTOK], logical
            .reshape(KD, 128, NBT, TILE_T)
            .transpose(2, 1, 0, 3)                  # [j, p, k, t]
            .reshape(NBT, 128, KD * TILE_T))
        enc_i = ((1.0 / 32.0) * encoding[sl].T.reshape(KD, 128, B_LOC)
                 .transpose(1, 0, 2).reshape(128, KD * B_LOC))
        mperm = (np.ascontiguousarray(mask[sl]).reshape(NTOK)
                 .reshape(NBT, 4, 4, 32).transpose(0, 2, 1, 3)
                 .reshape(NTOK))
        mask_i = mperm.reshape(NTOK // 128, 128).T
        consts_i = np.zeros((128, CW), np.float32)
        consts_i[:, 0:KD * B_LOC] = enc_i
        for eb in range(EB):
            consts_i[:, KD * B_LOC + 32 * eb] = w_a[eb * 128:(eb + 1) * 128]
        o0 = KD * B_LOC + EB * 32
        consts_i[:, o0] = 1.0
        consts_i[[0, 32, 64, 96], o0 + 1] = 1.0
        consts_i[:, o0 + 2:o0 + 2 + NTOK // 128] = mask_i
        in_maps.append({
            "x": x_i, "xt": xt_i, "wyt": wyt_rows, "wht": wht_rows,
            "consts": consts_i.astype(ml_dtypes.bfloat16),
        })
    return in_maps


def run(inputs, trace=False):
    if "nc" not in _CACHE:
        _CACHE["nc"] = build()
    nc = _CACHE["nc"]
    in_maps = _host_pack(**inputs)
    res = run_bass_kernel_spmd(nc, in_maps, core_ids=list(range(N_CORES)),
                               trace=trace)
    out = np.concatenate([res.results[i]["out"] for i in range(N_CORES)],
                         axis=0)
    return out, res


def kernel(**inputs):
    inputs = {k: np.asarray(v) for k, v in inputs.items()}
    out, _ = run(inputs, trace=False)
    return out


# revision 20
# speedup vs baseline: 1.2243x; 1.0161x over previous
"""Trainium2 Bass kernel v3 for masked additive-attention pooling.

Reference math (per batch b):
    whhn = encoding @ W_h.T                            # [B, D]
    M    = tanh(X @ W_y.T + whhn[:, None, :])          # [B, T, D]
    a    = sigmoid(M @ w_a)                            # [B, T]
    e    = exp(a); den = sum(e * mask); w = e * mask / den
    out  = sum_t w[t] * X[t]                           # [B, D]

Sharding: data-parallel over batch B=32 across 8 cores (4 batches/core).
Weights replicated. Host does layout transforms only.

v3 changes vs v2 (183us measured):
  - apre (logits): 4-way col-tiled (tile_position) N=256 matmuls, one
    [128,32] wa-chunk stationary per strip, accumulating 8 e-chunks into
    4 PSUM row-strips -> ~120ns/group vs 2 DR N=512 MMs; kills the DR
    apre stream (13.8us) AND the t_cols K=1 transpose matmuls (~8us).
  - strip output goes through ONE DVE 32x32 block-transpose per half
    ([128,256] PSUM -> SBUF); a host-side token permutation within each
    512-token j-tile (c<->k swap) makes the transposed layout line up
    exactly with the pooling stationary columns. x-natural and mask are
    permuted identically on host; xt/z-stream order unchanged.
  - th stored as plain [128, eb, jl, 512] fp8 (no DR pair interleave).
  - apre groups emitted inline one eb behind the z stream (ACT latency
    hidden), so the epilogue tail after the last z group is short.
  - DMA issue spread across sync/scalar/vector/gpsimd queues with
    first-needed-first ordering: head shrinks and the early z stream no
    longer starves (HAM stays warm).
  - z DR stream unchanged: ~110us of fp8 DoubleRow matmuls = the PE
    hardware floor for this problem.
"""

import sys

if "/opt/trn_rl_repo" not in sys.path:
    sys.path.insert(0, "/opt/trn_rl_repo")

import numpy as np
import ml_dtypes

import concourse.bacc as bacc
import concourse.mybir as mybir
import concourse.tile as tile
from concourse.bass_utils import run_bass_kernel_spmd

F32 = mybir.dt.float32
BF16 = mybir.dt.bfloat16
FP8 = mybir.dt.float8e4
AF = mybir.ActivationFunctionType
DR = mybir.MatmulPerfMode.DoubleRow
MULT = mybir.AluOpType.mult
ADD = mybir.AluOpType.add

N_CORES = 8
B, T, D = 32, 2048, 1024
B_LOC = B // N_CORES          # 4 batches per core
NTOK = B_LOC * T              # 8192 tokens per core
TILE_T = 512                  # tokens per j-tile
NBT = NTOK // TILE_T          # 16 j-tiles
BT_PER_B = T // TILE_T        # 4 j-tiles per batch
CH = TILE_T // 128            # 4 128-token chunks per j-tile
KD = D // 128                 # 8 contraction chunks
EB = D // 128                 # 8 output-feature blocks
NH = NBT // 2                 # 8 halves (j-pairs)

_CACHE = {}


def build():
    nc = bacc.Bacc("TRN2", target_bir_lowering=False, debug=False,
                   num_devices=N_CORES)

    x = nc.dram_tensor("x", [NTOK, D], BF16, kind="ExternalInput").ap()
    xt = nc.dram_tensor("xt", [NBT, 128, KD * TILE_T], FP8,
                        kind="ExternalInput").ap()
    wyt = nc.dram_tensor("wyt", [EB, 128, KD * 128], FP8,
                         kind="ExternalInput").ap()
    wht = nc.dram_tensor("wht", [EB, 128, KD * 128], FP8,
                         kind="ExternalInput").ap()
    CW = KD * B_LOC + EB * 32 + 1 + 1 + NTOK // 128
    consts = nc.dram_tensor("consts", [128, CW], BF16,
                            kind="ExternalInput").ap()
    out = nc.dram_tensor("out", [B_LOC, D], F32, kind="ExternalOutput").ap()

    x4 = x.rearrange("(j c p) d -> j p c d", p=128, c=CH)

    with tile.TileContext(nc) as tc:
        with tc.tile_pool(name="consts", bufs=1) as cp, \
             tc.tile_pool(name="wy", bufs=1) as wyp, \
             tc.tile_pool(name="xnat", bufs=8) as xp, \
             tc.tile_pool(name="xt", bufs=4) as xtp, \
             tc.tile_pool(name="th", bufs=2) as thp, \
             tc.tile_pool(name="small", bufs=2) as smp, \
             tc.tile_pool(name="mps", bufs=1, space="PSUM") as psum:

            state = {}
            pending = []

            def pop1():
                if pending:
                    pending.pop(0)()

            def load_xt(j, split=1, eng=None):
                eng = eng or nc.gpsimd
                t = xtp.tile([128, KD * TILE_T], FP8, tag="xt",
                             name=f"xt_{j}")
                w = KD * TILE_T // split
                for s in range(split):
                    eng.dma_start(
                        t[:, s * w:(s + 1) * w],
                        xt[j][:, s * w:(s + 1) * w])
                state[("xt", j)] = t

            def load_xnat(j, eng=None):
                eng = eng or nc.sync
                t = xp.tile([128, CH * D], BF16, tag="xn", name=f"x_{j}")
                eng.dma_start(
                    t[:].rearrange("p (c d) -> p c d", c=CH), x4[j])
                state[("xn", j)] = t

            # ---- phase 0: DMAs spread across queues, first-needed-first.
            whp_cm = tc.tile_pool(name="wh", bufs=1)
            whp = whp_cm.__enter__()
            xt_t0 = xtp.tile([128, KD * TILE_T], FP8, tag="xt", name="xt_0")
            xt_t1 = xtp.tile([128, KD * TILE_T], FP8, tag="xt", name="xt_1")
            state[("xt", 0)] = xt_t0
            state[("xt", 1)] = xt_t1
            wy_sb = [wyp.tile([128, KD * 128], FP8, tag=f"wy{eb}",
                              name=f"wy_{eb}") for eb in range(EB)]
            wh_sb = [whp.tile([128, KD * 128], FP8, tag=f"wh{eb}",
                              name=f"wh_{eb}") for eb in range(EB)]
            consts_sb = cp.tile([128, CW], BF16)
            enc_sb = consts_sb[:, 0:KD * B_LOC]
            wa_sb = [consts_sb[:, KD * B_LOC + 32 * eb:
                               KD * B_LOC + 32 * (eb + 1)]
                     for eb in range(EB)]
            _o0 = KD * B_LOC + EB * 32
            ones_sb = consts_sb[:, _o0:_o0 + 1]
            hot_sb = consts_sb[:, _o0 + 1:_o0 + 2]
            mask_sb = consts_sb[:, _o0 + 2:_o0 + 2 + NTOK // 128]
            half_sb = cp.tile([128, 1], F32)
            nc.vector.memset(half_sb[:], 0.5)
            whhn_sb = cp.tile([128, EB * B_LOC], F32)

            # sync queue: xt0/xt1 quarters (the z-stream critical path)
            QW = KD * TILE_T // 4
            for s in range(4):
                nc.sync.dma_start(xt_t0[:, s * QW:(s + 1) * QW],
                                  xt[0][:, s * QW:(s + 1) * QW])
                nc.sync.dma_start(xt_t1[:, s * QW:(s + 1) * QW],
                                  xt[1][:, s * QW:(s + 1) * QW])
            # scalar queue: wy weights (one needed every ~2.1us), then the
            # first x-natural tiles (needed at h0's pooling, ~+18us)
            for s in range(EB):
                nc.scalar.dma_start(wy_sb[s][:], wyt[s])
            load_xnat(0, eng=nc.scalar)
            load_xnat(1, eng=nc.scalar)
            # gpsimd queue: ONE packed consts DMA, all wh (h0-critical),
            # then the h1 xt tiles
            nc.gpsimd.dma_start(consts_sb[:], consts[:])
            for s in range(EB):
                nc.gpsimd.dma_start(wh_sb[s][:], wht[s])
            load_xt(2, eng=nc.gpsimd)
            load_xt(3, eng=nc.gpsimd)
            load_xnat(2, eng=nc.sync)
            load_xnat(3, eng=nc.sync)

            # ---- whhn: one eb at a time (interleaved into early z) ----
            def emit_whhn(eb):
                def fn():
                    php = psum.tile([128, B_LOC], F32, tag="small", bufs=1,
                                    name=f"php_{eb}")
                    for k in range(KD):
                        nc.tensor.matmul(
                            php[:], wh_sb[eb][:, k * 128:(k + 1) * 128],
                            enc_sb[:, k * B_LOC:(k + 1) * B_LOC],
                            start=(k == 0), stop=(k == KD - 1))
                    nc.vector.tensor_copy(
                        whhn_sb[:, eb * B_LOC:(eb + 1) * B_LOC], php[:])
                return fn

            # ---- z + tanh for one (half, eb): both j-tiles share LDW ----
            def get_th(h):
                key = ("th", h)
                if key not in state:
                    state[key] = thp.tile([128, EB * 2 * 512], FP8,
                                          tag="th", name=f"th_{h}")
                return state[key]

            def emit_z(h, eb, mid=None):
                j0 = 2 * h
                b = j0 // BT_PER_B
                zp = psum.tile([128, 2 * 512], F32, tag="z", bufs=2,
                               name=f"z_{h}_{eb}")
                xt0 = state[("xt", j0)]
                xt1 = state[("xt", j0 + 1)]
                for q in range(KD // 2):
                    w_ap = wy_sb[eb][:, q * 256:(q + 1) * 256].rearrange(
                        "p (i m) -> p i m", i=2)
                    for jl, xt_t in ((0, xt0), (1, xt1)):
                        mm = nc.tensor.matmul(
                            zp[:, jl * 512:(jl + 1) * 512], w_ap,
                            xt_t[:, 2 * q * TILE_T:(2 * q + 2) * TILE_T]
                            .rearrange("p (i n) -> p i n", i=2),
                            start=(q == 0), stop=(q == KD // 2 - 1),
                            perf_mode=DR)
                        if jl == 1:
                            mm.ldweights = False
                if mid is not None:
                    mid()
                # tanh over both j-tiles at once; bias whhn[:, eb, b]
                th_t = get_th(h)
                thv = th_t[:].rearrange("p (e jl n) -> p e jl n",
                                        e=EB, jl=2)
                nc.scalar.activation(
                    thv[:, eb], zp[:].rearrange("p (jl n) -> p jl n", jl=2),
                    AF.Tanh, scale=1.0 / 32.0,
                    bias=whhn_sb[:, eb * B_LOC + b:eb * B_LOC + b + 1])

            # ---- apre: 4-way col-tiled strips, one group per (h, eb).
            # Full-bank tile (exclusive has_written domain): the single
            # start=True on the first MM clears the bank; the other strips
            # first-touch-overwrite, then everything accumulates.
            def get_aps(h):
                key = ("aps", h)
                if key not in state:
                    t = psum.tile([128, 512], F32, tag="aps",
                                  bufs=1, name=f"aps_{h}")
                    nc.vector.memset(t[:, 0:256], 0.0)
                    state[key] = t
                return state[key]

            def emit_apre(h, eb):
                th_t = state[("th", h)]
                thv = th_t[:].rearrange("p (e jl n) -> p e jl n",
                                        e=EB, jl=2)
                aps = get_aps(h)
                apsv = aps[:, 0:256].rearrange("p (jl n) -> p jl n", jl=2)
                for c in range(4):
                    nc.tensor.matmul(
                        apsv[32 * c:32 * (c + 1), :, :],
                        wa_sb[eb],
                        thv[:, eb, :, 128 * c:128 * (c + 1)],
                        start=False, stop=(eb == EB - 1),
                        tile_position=(0, 32 * c),
                        skip_group_check=True)
                if eb == EB - 1:
                    state.pop(("th", h))

            # ---- per-half epilogue, interleaved between z groups ----
            def get_ew(b):
                key = ("ew", b)
                if key not in state:
                    state[key] = smp.tile([128, 4 * CH], BF16, tag="ew",
                                          name=f"ew_{b}")
                return state[key]

            def queue_h_epilogue(h):
                b = h // 2
                hh = h % 2       # which j-pair of the batch

                def t_trans():
                    aps = state.pop(("aps", h))
                    apct = smp.tile([128, 256], F32, tag="apct",
                                    name=f"apct_{h}")
                    nc.vector.transpose(apct[:], aps[:, 0:256])
                    state[("apct", h)] = apct

                def t_act():
                    apct = state.pop(("apct", h))
                    apcv = apct[:].rearrange("p (m e) -> p m e", m=8)
                    tj = smp.tile([128, 8], F32, tag="tj", name=f"tj_{h}")
                    nc.scalar.activation(tj[:], apcv[:, :, 0:1], AF.Tanh,
                                         scale=0.5)
                    ej = smp.tile([128, 8], F32, tag="ej", name=f"ej_{h}")
                    nc.scalar.activation(ej[:], tj[:], AF.Exp,
                                         bias=half_sb[:], scale=0.5)
                    ew = get_ew(b)
                    nc.vector.tensor_mul(
                        ew[:, hh * 8:(hh + 1) * 8], ej[:],
                        mask_sb[:, b * 16 + hh * 8:b * 16 + (hh + 1) * 8])

                def t_den():
                    ew = state[("ew", b)]
                    acc = smp.tile([128, 1], F32, tag="acc",
                                   name=f"acc_{b}")
                    nc.vector.tensor_reduce(
                        acc[:], ew[:], mybir.AxisListType.XYZW, ADD)
                    accb = smp.tile([128, 1], BF16, tag="accb",
                                    name=f"accb_{b}")
                    nc.vector.tensor_copy(accb[:], acc[:])
                    den = psum.tile([1, 1], F32, tag="small", bufs=1,
                                    name=f"den_{b}")
                    nc.tensor.matmul(den[0:1, :], ones_sb,
                                     accb[:], start=True, stop=True)
                    rec = smp.tile([1, 1], F32, tag="rec", name=f"rec_{b}")
                    nc.vector.reciprocal(rec[:], den[0:1, :])
                    state[("rec", b)] = rec

                def t_pool(dn):
                    def fn():
                        ew = state[("ew", b)]
                        if hh == 0:
                            state[("num", b, dn)] = psum.tile(
                                [128, 512], F32, tag="num", bufs=2,
                                name=f"num_{b}_{dn}")
                            nc.vector.memset(state[("num", b, dn)][:], 0.0)
                        num = state[("num", b, dn)]
                        for jl in range(2):
                            j = 2 * h + jl
                            xn = state[("xn", j)]
                            for k in range(CH):
                                col = (hh * 2 + jl) * CH + k
                                pos = k * 32
                                nc.tensor.matmul(
                                    num[pos:pos + 1, :],
                                    ew[:, col:col + 1],
                                    xn[:, k * D + dn * 512:
                                       k * D + (dn + 1) * 512],
                                    start=(col == 0),
                                    stop=(hh == 1 and jl == 1),
                                    tile_position=(0, pos),
                                    skip_group_check=True)
                            if dn == 1:
                                state.pop(("xn", j))
                    return fn

                def t_scale(dn):
                    def fn():
                        num = state.pop(("num", b, dn))
                        nsb = smp.tile([128, 512], BF16, tag="nsb",
                                       name=f"nsb_{b}_{dn}")
                        nc.vector.tensor_copy(nsb[:], num[:])
                        ns = psum.tile([1, 512], F32, tag="small", bufs=1,
                                       name=f"ns_{b}_{dn}")
                        nc.tensor.matmul(ns[0:1, :], hot_sb, nsb[:],
                                         start=True, stop=True)
                        rec = state[("rec", b)]
                        if ("ob", b) not in state:
                            state[("ob", b)] = smp.tile(
                                [1, D], F32, tag="ob", name=f"ob_{b}")
                        ob = state[("ob", b)]
                        nc.vector.tensor_scalar_mul(
                            ob[:, dn * 512:(dn + 1) * 512], ns[0:1, :],
                            rec[:])
                        if dn == 1:
                            state.pop(("rec", b))
                            state.pop(("ew", b))
                            ob = state.pop(("ob", b))
                            nc.sync.dma_start(out[b:b + 1, :], ob[:])
                    return fn

                pending.append(t_trans)
                pending.append(t_act)
                if hh == 1:
                    pending.append(t_den)
                pending.append(t_pool(0))
                pending.append(t_pool(1))
                if hh == 1:
                    pending.append(t_scale(0))
                    pending.append(t_scale(1))

            # ---- main loop over halves ----
            for h in range(NH):
                j0 = 2 * h
                if j0 + 4 < NBT:
                    load_xt(j0 + 4)
                if j0 + 5 < NBT:
                    load_xt(j0 + 5)
                if j0 + 4 < NBT:
                    load_xnat(j0 + 4)
                if j0 + 5 < NBT:
                    load_xnat(j0 + 5)
                for eb in range(EB):
                    if h == 0:
                        emit_z(h, eb, mid=emit_whhn(eb))
                    else:
                        emit_z(h, eb)
                    # apre one group behind the z stream (ACT latency)
                    if eb > 0:
                        emit_apre(h, eb - 1)
                    elif h > 0:
                        emit_apre(h - 1, EB - 1)
                    if h > 0:
                        pop1()
                        if len(pending) > 5:
                            pop1()
                queue_h_epilogue(h)
                if h == 0:
                    whp_cm.__exit__(None, None, None)
            emit_apre(NH - 1, EB - 1)
            while pending:
                pop1()

    nc.compile()
    return nc


def _host_pack(full_input, encoding, mask, W_h, W_y, w_a):
    """Per-core input maps (layout transforms / casts only)."""
    wyT = np.ascontiguousarray(W_y.T)  # [d, e]
    whT = np.ascontiguousarray(W_h.T)
    wyt_rows = np.empty((EB, 128, KD * 128), ml_dtypes.float8_e4m3)
    wht_rows = np.empty((EB, 128, KD * 128), ml_dtypes.float8_e4m3)
    for eb in range(EB):
        for k in range(KD):
            wyt_rows[eb, :, k * 128:(k + 1) * 128] = (
                32.0 * wyT[k * 128:(k + 1) * 128, eb * 128:(eb + 1) * 128])
            wht_rows[eb, :, k * 128:(k + 1) * 128] = (
                32.0 * whT[k * 128:(k + 1) * 128, eb * 128:(eb + 1) * 128])
    CW = KD * B_LOC + EB * 32 + 1 + 1 + NTOK // 128

    in_maps = []
    for ci in range(N_CORES):
        sl = slice(ci * B_LOC, (ci + 1) * B_LOC)
        xf = np.ascontiguousarray(
            full_input[sl].reshape(NTOK, D).astype(np.float32))
        # token permutation within each j-tile: stored (k, cc, i) holds
        # logical (cc, k, i) so the DVE block transpose of the apre strips
        # lands exactly on the pooling stationary layout.
        xperm = np.ascontiguousarray(
            xf.reshape(NBT, 4, 4, 32, D).transpose(0, 2, 1, 3, 4)
            .reshape(NTOK, D))
        x_i = xperm.astype(ml_dtypes.bfloat16)
        xt_i = np.ascontiguousarray(
            xf.T.astype(ml_dtypes.float8_e4m3)      # [D, NTOK], logical
            .reshape(KD, 128, NBT, TILE_T)
            .transpose(2, 1, 0, 3)                  # [j, p, k, t]
            .reshape(NBT, 128, KD * TILE_T))
        enc_i = ((1.0 / 32.0) * encoding[sl].T.reshape(KD, 128, B_LOC)
                 .transpose(1, 0, 2).reshape(128, KD * B_LOC))
        mperm = (np.ascontiguousarray(mask[sl]).reshape(NTOK)
                 .reshape(NBT, 4, 4, 32).transpose(0, 2, 1, 3)
                 .reshape(NTOK))
        mask_i = mperm.reshape(NTOK // 128, 128).T
        consts_i = np.zeros((128, CW), np.float32)
        consts_i[:, 0:KD * B_LOC] = enc_i
        for eb in range(EB):
            consts_i[:, KD * B_LOC + 32 * eb] = w_a[eb * 128:(eb + 1) * 128]
        o0 = KD * B_LOC + EB * 32
        consts_i[:, o0] = 1.0
        consts_i[[0, 32, 64, 96], o0 + 1] = 1.0
        consts_i[:, o0 + 2:o0 + 2 + NTOK // 128] = mask_i
        in_maps.append({
            "x": x_i, "xt": xt_i, "wyt": wyt_rows, "wht": wht_rows,
            "consts": consts_i.astype(ml_dtypes.bfloat16),
        })
    return in_maps


def run(inputs, trace=False):
    if "nc" not in _CACHE:
        _CACHE["nc"] = build()
    nc = _CACHE["nc"]
    in_maps = _host_pack(**inputs)
    res = run_bass_kernel_spmd(nc, in_maps, core_ids=list(range(N_CORES)),
                               trace=trace)
    out = np.concatenate([res.results[i]["out"] for i in range(N_CORES)],
                         axis=0)
    return out, res


def kernel(**inputs):
    inputs = {k: np.asarray(v) for k, v in inputs.items()}
    out, _ = run(inputs, trace=False)
    return out
